# revision 1
# baseline (speedup 1.0000x reference)
"""Trainium2 Bass kernel for EqualizedModConv2d (StyleGAN-style modulated conv).

Math (per sample b):
    s[b,i]   = sqrt(2) * leaky_relu(y[b] @ (mod_weight * dlat^-0.5).T + bias, 0.2)
    ww       = weight * s[b,:]                       (modulate, per in-channel)
    d[b,o]   = rsqrt(sum_{i,kh,kw} ww^2 + eps)       (demodulate, per out-channel)
    out[b]   = d[b,:] * conv2d(x[b], weight * s[b,:], SAME)

Factorization used on device (identical math):
    out[b,o,:] = d[b,o] * sum_{t,i} W_t[i,o] * (s[b,i] * xpad[b,i,:shifted(t)])
so the big conv runs with the *shared raw* weight as 9 accumulated matmul taps,
and the modulation/demodulation become tiny per-partition broadcast scalings of
the conv input/output. The O(B*C) style vectors s and d are computed on host
(0.01% of FLOPs); all heavy compute (38.7 GFLOP conv) runs on the NeuronCores.

Sharding: data-parallel over batch, 1 sample per NeuronCore across 8 cores.
The conv weight is broadcast (replicated) to all cores; per-core inputs are the
sample's image plus its style/demod vectors.

Device kernel shape (per core): the conv runs as 288 accumulated PE matmuls
(lhsT [128,128] x rhs [128,512], float32r = 1 cycle/row) into 8 PSUM banks
((C_out chunk) x (pixel half)); weights stream from HBM paced ahead of PE
consumption; throwaway warm-up matmuls on a broadcast zero-constant hold the
PE busy through the DMA-latency prologue so the HAM clock-gate is at 2.4 GHz
when the real stream starts; the last input-channel chunk drains group-by-
group so demod + output stores overlap the remaining matmuls. Tile-cost-model
estimate: ~68.4 us/core, a gap-free PE schedule at ~90% of the ~61.4 us f32r
compute roofline (3.1 us DMA-latency prologue + 61.4 us dense PE + 3.9 us
drain, the last two mostly fixed HW/Tile latencies). Measured hardware
relative error vs the fp32 reference: ~1.5e-4 (float32r is tf32-like reduced
precision at full PE rate; exact-fp32 matmul would run at 1/4 rate).
"""

import hashlib
import numpy as np
from contextlib import ExitStack

import concourse.mybir as mybir
import concourse.tile as tile
from concourse import bacc

# Problem shapes (hardcoded per contract).
B, C_IN, C_OUT, H, W, KS, DLAT = 8, 512, 512, 32, 32, 3, 512
HP, WP = H + 2, W + 2            # zero-padded input plane: 34 x 34
NPIX = H * W                     # 1024 (unpadded, as transferred)
PCH_I = C_IN // 128              # 4 input-channel chunks of 128 partitions
PCH_O = C_OUT // 128             # 4 output-channel chunks
NTAPS = KS * KS                  # 9 kernel taps
NW = NTAPS * PCH_I               # 36 lhsT tiles of [128, C_OUT]
NHALF = 2                        # output pixels split into 2 PSUM banks of 512
HHALF = H // NHALF               # 16 output rows per half
N_CORES = 8

SQRT2 = 1.4142135623730951
LRELU_SLOPE = 0.2
EPS = 1e-8

# Matmul precision: "f32r" (fp32 storage, reduced-precision PE pass, 1 cyc/row),
# "bf16" (cast inputs, 1 cyc/row), "f32" (exact, 4 cyc/row).
MM_DTYPE = "f32r"

# Throwaway matmuls issued during the DMA prologue to hold the PE busy, so
# the HAM clock-gate un-throttles (1.2 -> 2.4 GHz) before the real conv
# stream begins. PE is otherwise idle for the first ~3.5us.
N_WARMUP = 6

# 1-D Winograd F(2,3) along W: 192 matmuls instead of 288 (PE 41us vs 61us)
# at the cost of DVE input/output transforms (exact +-1/+-0.5 constants).
# HW-validated (rel err 1.90e-4 vs the fp32 reference; cost model 56.2us/core
# vs 68.4us direct). NOTE: the drain must read PSUM only via tensor_copy in
# operand-0 position — tensor_tensor with a PSUM second operand hard-faults
# the exec unit (NRT_EXEC_UNIT_UNRECOVERABLE, found the hard way).
WINOGRAD = True
WT = W // 2                      # 16 transform tiles per row

_NC_CACHE: dict = {}
_RUNNER_CACHE: dict = {}
_W_DEV_CACHE: dict = {}


def _mm_dts(mm_dtype):
    if mm_dtype == "f32r":
        return mybir.dt.float32r
    if mm_dtype == "bf16":
        return mybir.dt.bfloat16
    return mybir.dt.float32


def _emit_conv(ctx, tc, o_d, w_d, x_d, sdm_d, mm_dtype):
    nc = tc.nc
    f32 = mybir.dt.float32
    mm_dt = _mm_dts(mm_dtype)

    singles = ctx.enter_context(tc.tile_pool(name="singles", bufs=1))
    psum = ctx.enter_context(tc.tile_pool(name="psum", bufs=8, space="PSUM"))
    outp = ctx.enter_context(tc.tile_pool(name="outp", bufs=4))

    # Input planes: DMA unpadded; zero only the pad border (GpSimd — otherwise
    # idle), then modulate the interior: xs = s * x.
    x_sb = singles.tile([128, PCH_I, H, W], f32)
    sdm_sb = singles.tile([128, PCH_I + PCH_O], f32)
    xs_sb = singles.tile([128, PCH_I, HP, WP], mm_dt)
    xs_z = xs_sb.bitcast(f32) if mm_dtype == "f32r" else xs_sb
    for c in range(PCH_I):
        nc.gpsimd.memset(xs_z[:, c, 0, :], 0.0)
        nc.gpsimd.memset(xs_z[:, c, HP - 1, :], 0.0)
        nc.gpsimd.memset(xs_z[:, c, 1 : HP - 1, 0:1], 0.0)
        nc.gpsimd.memset(xs_z[:, c, 1 : HP - 1, WP - 1 : WP], 0.0)

    # Conv weights in lhsT layout, in-chunk-major: ti = kc*9 + tap.
    # First chunk arrives as 9 small per-tap DMAs (PE can start ~immediately);
    # remaining chunks as 3 large block DMAs (cheap to issue, arrive in time).
    w_sb = singles.tile([128, NW, C_OUT], mm_dt)

    def x_load(c):
        nc.sync.dma_start(x_sb[:, c, :, :], x_d[c].rearrange("p (h w) -> p h w", h=H))
        nc.vector.tensor_scalar_mul(
            xs_sb[:, c, 1 : H + 1, 1 : W + 1], x_sb[:, c, :, :], sdm_sb[:, c : c + 1]
        )

    # Interleave so the critical chain (x0 top half -> scale -> first matmul)
    # issues first. The tiny s/d vector DMA goes via GpSimd's SWDGE so it
    # doesn't occupy a slot in the SP issue chain; w taps are issued early
    # enough that PE's consumption order never outruns arrival.
    nc.gpsimd.dma_start(sdm_sb[:, :], sdm_d[:, :])
    x0v = x_d[0].rearrange("p (h w) -> p h w", h=H)
    nc.sync.dma_start(x_sb[:, 0, 0:HHALF, :], x0v[:, 0:HHALF, :])
    # First tap split: the very first matmul only needs lhsT cols 0:128, so a
    # quarter-DMA unblocks the real stream ~0.5us sooner.
    nc.sync.dma_start(w_sb[:, 0, 0:128], w_d[0, 0][:, 0:128])
    nc.sync.dma_start(w_sb[:, 0, 128:C_OUT], w_d[0, 0][:, 128:C_OUT])
    nc.vector.tensor_scalar_mul(
        xs_sb[:, 0, 1 : HHALF + 1, 1 : W + 1],
        x_sb[:, 0, 0:HHALF, :],
        sdm_sb[:, 0:1],
    )
    nc.sync.dma_start(x_sb[:, 0, HHALF:H, :], x0v[:, HHALF:H, :])
    nc.vector.tensor_scalar_mul(
        xs_sb[:, 0, HHALF + 1 : H + 1, 1 : W + 1],
        x_sb[:, 0, HHALF:H, :],
        sdm_sb[:, 0:1],
    )
    nc.sync.dma_start(w_sb[:, 1, :], w_d[0, 1])
    x_load(1)
    nc.sync.dma_start(w_sb[:, 2, :], w_d[0, 2])
    nc.sync.dma_start(w_sb[:, 3, :], w_d[0, 3])
    x_load(2)
    nc.sync.dma_start(w_sb[:, 4, :], w_d[0, 4])
    nc.sync.dma_start(w_sb[:, 5, :], w_d[0, 5])
    x_load(3)
    for t9 in range(6, NTAPS):
        nc.sync.dma_start(w_sb[:, t9, :], w_d[0, t9])
    # kc=1 lands closest to PE's consumption point — split it so its first
    # taps arrive before PE drains chunk 0; kc=2/3 have plenty of slack.
    nc.sync.dma_start(
        w_sb[:, NTAPS : NTAPS + 4, :],
        w_d[1, 0:4].rearrange("t p n -> p t n"),
    )
    nc.sync.dma_start(
        w_sb[:, NTAPS + 4 : 2 * NTAPS, :],
        w_d[1, 4:NTAPS].rearrange("t p n -> p t n"),
    )
    for kc in range(2, PCH_I):
        nc.sync.dma_start(
            w_sb[:, kc * NTAPS : (kc + 1) * NTAPS, :],
            w_d[kc].rearrange("t p n -> p t n"),
        )

    # 8 accumulation groups: (out-chunk mc, pixel-half nh), one PSUM bank each.
    ps = [[psum.tile([128, HHALF * W], f32, name="ps", tag="ps") for _ in range(NHALF)]
          for _ in range(PCH_O)]

    # PE warm-up into the first PSUM bank (each a complete start/stop group,
    # so the real accumulation's start=True wipes them). Streams a broadcast
    # view of the preamble zero-constant, so it has no data dependencies and
    # fills the DMA-latency prologue from t~0.
    if N_WARMUP:
        wz = nc.const_aps.tensor(0.0, [128, 1], f32)
        if mm_dtype == "f32r":
            wz = wz.bitcast(mm_dt)
        elif mm_dtype == "bf16":
            wz = nc.const_aps.tensor(0.0, [128, 1], mm_dt)
        for _ in range(N_WARMUP):
            nc.tensor.matmul(
                ps[0][0][:, :],
                lhsT=wz.to_broadcast([128, 128]),
                rhs=wz.to_broadcast([128, HHALF * W]),
                start=True,
                stop=True,
            )

    def mm(mc, nh, kc, t9, start, stop):
        kh, kw = divmod(t9, KS)
        r0 = nh * HHALF + kh
        nc.tensor.matmul(
            ps[mc][nh][:, :],
            lhsT=w_sb[:, kc * NTAPS + t9, mc * 128 : (mc + 1) * 128],
            rhs=xs_sb[:, kc, r0 : r0 + HHALF, kw : kw + W],
            start=start,
            stop=stop,
        )

    # Phase A: chunks 0..2 in weight-arrival order, all 8 groups in parallel
    # (PE stays dense and consumes w tiles as the DMAs land). Within chunk 0,
    # the top-half (nh=0) matmuls of taps kh<2 depend only on the first half
    # of x0, so they go first — PE starts while the rest of x still streams.
    started = set()

    def mm_tracked(mc, nh, kc, t9):
        key = (mc, nh)
        mm(mc, nh, kc, t9, start=key not in started, stop=False)
        started.add(key)

    order0 = [(t9, mc, 0) for t9 in range(2 * KS) for mc in range(PCH_O)]
    seen0 = set(order0)
    order0 += [
        (t9, mc, nh)
        for t9 in range(NTAPS)
        for mc in range(PCH_O)
        for nh in range(NHALF)
        if (t9, mc, nh) not in seen0
    ]
    for t9, mc, nh in order0:
        mm_tracked(mc, nh, 0, t9)
    for kc in range(1, PCH_I - 1):
        for t9 in range(NTAPS):
            for mc in range(PCH_O):
                for nh in range(NHALF):
                    mm_tracked(mc, nh, kc, t9)

    # Phase B: last chunk group-by-group so groups finish staggered ~1.9us
    # apart and the demod+store drains overlap with remaining PE work.
    kc = PCH_I - 1

    for mc in range(PCH_O):
        for nh in range(NHALF):
            ob = outp.tile([128, HHALF * W], f32, name="ob", tag="ob")
            dm_col = sdm_sb[:, PCH_I + mc : PCH_I + mc + 1]
            o_slice = o_d[
                mc * 128 : (mc + 1) * 128, nh * HHALF * W : (nh + 1) * HHALF * W
            ]
            for t9 in range(NTAPS):
                mm(mc, nh, kc, t9, start=False, stop=(t9 == NTAPS - 1))
            nc.vector.tensor_scalar_mul(ob[:, :], ps[mc][nh][:, :], dm_col)
            nc.sync.dma_start(o_slice, ob[:, :])


def _emit_conv_wino(ctx, tc, o_d, w_d, x_d, sdm_d, mm_dtype):
    """1-D Winograd F(2,3) along W. Transform-domain conv per point xi:
    Y_xi = sum_{kh,i} Wt[xi,kh][i,o] * X_xi[i, h+kh, t]; outputs combine as
    y[2t] = (Y0+Y1+Y2)*d, y[2t+1] = (Y1-Y2-Y3)*d. 16 (mc,xi) accumulation
    groups run as two waves of 8 over output-channel pairs so each wave's
    drain has all 4 transform points resident."""
    nc = tc.nc
    f32 = mybir.dt.float32
    mm_dt = _mm_dts(mm_dtype)

    singles = ctx.enter_context(tc.tile_pool(name="singles", bufs=1))
    xpool = ctx.enter_context(tc.tile_pool(name="xpool", bufs=2))
    psum = ctx.enter_context(tc.tile_pool(name="psum", bufs=8, space="PSUM"))
    outp = ctx.enter_context(tc.tile_pool(name="outp", bufs=2))
    tmps = ctx.enter_context(tc.tile_pool(name="tmps", bufs=4))

    sdm_sb = singles.tile([128, PCH_I + 2 * PCH_O], f32)
    nc.gpsimd.dma_start(sdm_sb[:, :], sdm_d[:, :])

    # Transform-domain input X[c, xi, padded-row, tile] and weights
    # Wt[(kc,kh), xi, o].
    X_sb = singles.tile([128, PCH_I, 4, HP, WT], mm_dt)
    w_sb = singles.tile([128, PCH_I * KS, 4, C_OUT], mm_dt)

    def x_load(c):
        # Stage the chunk, modulate by s into the zero-bordered plane, then
        # compute the four B^T d column combinations (all +-1 adds).
        xst = xpool.tile([128, H, W], f32, name="xst", tag="xst")
        xs = xpool.tile([128, HP, WP], f32, name="xs", tag="xs")
        nc.gpsimd.memset(xs[:, 0, :], 0.0)
        nc.gpsimd.memset(xs[:, HP - 1, :], 0.0)
        nc.gpsimd.memset(xs[:, 1 : HP - 1, 0:1], 0.0)
        nc.gpsimd.memset(xs[:, 1 : HP - 1, WP - 1 : WP], 0.0)
        nc.sync.dma_start(xst[:, :, :], x_d[c].rearrange("p (h w) -> p h w", h=H))
        nc.vector.tensor_scalar_mul(
            xs[:, 1 : H + 1, 1 : W + 1], xst[:, :, :], sdm_sb[:, c : c + 1]
        )

        def dv(a):
            # xs cols a, a+2, ..., a+30 -> [128, HP, WT]
            base, par = (a, 0) if a % 2 == 0 else (a - 1, 1)
            return xs[:, :, base : base + 2 * WT].rearrange(
                "p h (t two) -> p h t two", two=2
            )[:, :, :, par]

        d0, d1, d2, d3 = dv(0), dv(1), dv(2), dv(3)
        nc.vector.tensor_sub(X_sb[:, c, 0], d0, d2)
        nc.vector.tensor_add(X_sb[:, c, 1], d1, d2)
        nc.vector.tensor_sub(X_sb[:, c, 2], d2, d1)
        nc.vector.tensor_sub(X_sb[:, c, 3], d1, d3)

    # Issue order: x chunks (SP) interleaved with per-(tap, xi) weight DMAs,
    # alternating SP / GpSimd queues so the two DGE paths stream in parallel
    # and PE's (t12, xi)-ordered consumption never outruns arrival.
    def w_load(t12, xi, eng):
        kc, kh = divmod(t12, KS)
        eng.dma_start(w_sb[:, t12, xi, :], w_d[kc, kh, xi])

    x_load(0)
    for xi in range(4):
        w_load(0, xi, nc.sync)
        w_load(1, xi, nc.gpsimd)
    x_load(1)
    for xi in range(4):
        w_load(2, xi, nc.gpsimd)
    x_load(2)
    for xi in range(4):
        w_load(3, xi, nc.sync)
        w_load(4, xi, nc.gpsimd)
    x_load(3)
    for t12 in range(5, PCH_I * KS):
        eng = nc.sync if t12 % 2 else nc.gpsimd
        for xi in range(4):
            w_load(t12, xi, eng)

    NT12 = PCH_I * KS            # 12 accumulation taps per group
    first_group = [True]

    for mcs in ((0, 1), (2, 3)):
        ps = {}
        for mc in mcs:
            for xi in range(4):
                ps[(mc, xi)] = psum.tile([128, H, WT], f32, name="ps", tag="ps")
        if first_group[0] and N_WARMUP:
            wz = nc.const_aps.tensor(0.0, [128, 1], f32)
            if mm_dtype == "f32r":
                wz = wz.bitcast(mm_dt)
            elif mm_dtype == "bf16":
                wz = nc.const_aps.tensor(0.0, [128, 1], mm_dt)
            for _ in range(N_WARMUP):
                nc.tensor.matmul(
                    ps[(mcs[0], 0)][:, :, :],
                    lhsT=wz.to_broadcast([128, 128]),
                    rhs=wz.to_broadcast([128, H * WT]),
                    start=True,
                    stop=True,
                )
            first_group[0] = False

        def mmw(mc, xi, t12, start, stop):
            kc, kh = divmod(t12, KS)
            nc.tensor.matmul(
                ps[(mc, xi)][:, :, :],
                lhsT=w_sb[:, t12, xi, mc * 128 : (mc + 1) * 128],
                rhs=X_sb[:, kc, xi, kh : kh + H, :],
                start=start,
                stop=stop,
            )

        # Early taps step-major (DMA-paced); late taps mc-major so the first
        # mc stops well before the second and its DVE drain hides completely
        # under the second mc's remaining matmuls. Wave B can stagger more
        # (all weights resident by then).
        N_STAG = 6 if mcs[0] == 0 else 9
        for t12 in range(NT12 - N_STAG):
            for mc in mcs:
                for xi in range(4):
                    mmw(mc, xi, t12, start=(t12 == 0), stop=False)
        for mc in mcs:
            for t12 in range(NT12 - N_STAG, NT12):
                for xi in range(4):
                    mmw(mc, xi, t12, start=False, stop=(t12 == NT12 - 1))
            # Drain this mc: A^T combine + demod, per h-half so stores of the
            # first half overlap the second half's DVE work.
            ob = outp.tile([128, H, W], f32, name="ob", tag="ob")
            dmc = sdm_sb[:, PCH_I + mc : PCH_I + mc + 1]
            ndmc = sdm_sb[:, PCH_I + PCH_O + mc : PCH_I + PCH_O + mc + 1]
            for h0 in (0, HHALF):
                Y = [ps[(mc, xi)][:, h0 : h0 + HHALF, :] for xi in range(4)]
                obv = ob[:, h0 : h0 + HHALF, :].rearrange(
                    "p h (t two) -> p h t two", two=2
                )
                # PSUM is only ever an operand-0 input (tensor_copy/add/sub
                # with the PSUM side first) — a PSUM operand in the in1 slot
                # hard-faults the exec unit. The odd-column chain computes
                # -(o) and the store scale uses -d to fix the sign.
                t1 = tmps.tile([128, HHALF, WT], f32, name="t1", tag="t1")
                et = tmps.tile([128, HHALF, WT], f32, name="et", tag="et")
                ot = tmps.tile([128, HHALF, WT], f32, name="ot", tag="ot")
                nc.vector.tensor_copy(t1[:, :, :], Y[1])
                nc.vector.tensor_add(et[:, :, :], Y[0], t1[:, :, :])
                nc.vector.tensor_add(et[:, :, :], Y[2], et[:, :, :])
                nc.vector.tensor_scalar_mul(obv[:, :, :, 0], et[:, :, :], dmc)
                nc.vector.tensor_sub(ot[:, :, :], Y[2], t1[:, :, :])
                nc.vector.tensor_add(ot[:, :, :], Y[3], ot[:, :, :])
                nc.vector.tensor_scalar_mul(obv[:, :, :, 1], ot[:, :, :], ndmc)
                nc.sync.dma_start(
                    o_d[mc * 128 : (mc + 1) * 128, h0 * W : (h0 + HHALF) * W],
                    ob[:, h0 : h0 + HHALF, :],
                )


def _build(mm_dtype, winograd=False):
    f32 = mybir.dt.float32
    w_io_dt = _mm_dts(mm_dtype) if mm_dtype != "f32" else f32
    nc = bacc.Bacc("TRN2", target_bir_lowering=False, debug=False)
    if winograd:
        w_d = nc.dram_tensor(
            "w", [PCH_I, KS, 4, 128, C_OUT], w_io_dt, kind="ExternalInput"
        ).ap()
    else:
        w_d = nc.dram_tensor(
            "w", [PCH_I, NTAPS, 128, C_OUT], w_io_dt, kind="ExternalInput"
        ).ap()
    x_d = nc.dram_tensor("x", [PCH_I, 128, NPIX], f32, kind="ExternalInput").ap()
    sdm_d = nc.dram_tensor(
        "sdm",
        [128, PCH_I + (2 if winograd else 1) * PCH_O],
        f32,
        kind="ExternalInput",
    ).ap()
    o_d = nc.dram_tensor("o", [C_OUT, H * W], f32, kind="ExternalOutput").ap()

    with tile.TileContext(nc) as tc:
        with ExitStack() as ctx:
            emit = _emit_conv_wino if winograd else _emit_conv
            emit(ctx, tc, o_d, w_d, x_d, sdm_d, mm_dtype)
    nc.compile()
    _strip_debug_info(nc)
    return nc


def _strip_debug_info(nc):
    """Null out source filenames/linenos/tracebacks in the BIR so its bytes —
    and therefore the NEFF compile-cache key — don't depend on where this
    file happens to live on disk."""
    for fn in nc.m.functions:
        for blk in fn.blocks:
            for inst in blk.instructions:
                try:
                    inst.debug = None
                except (AttributeError, TypeError):
                    pass
        for alloc in fn.allocations:
            # NOTE: alloc.debug (TensorDebugInfo) is path-free and CoreSim
            # needs it — only the per-memorylocation OpDebugInfo has paths.
            for ml in getattr(alloc, "memorylocations", None) or []:
                try:
                    ml.ant_debug = None
                except (AttributeError, TypeError):
                    pass


def get_nc(mm_dtype=MM_DTYPE, winograd=None):
    if winograd is None:
        winograd = WINOGRAD
    key = (mm_dtype, winograd)
    if key not in _NC_CACHE:
        _NC_CACHE[key] = _build(mm_dtype, winograd)
    return _NC_CACHE[key]


def prepare_inputs(x, y, weight, mod_weight, bias, mm_dtype=MM_DTYPE,
                   winograd=None):
    """Host-side prep: style/demod vectors + device data layouts.

    Returns a dict of *global* arrays: "w" replicated, others concatenated
    along axis 0 across the 8 cores (one sample per core).
    """
    x = np.ascontiguousarray(np.asarray(x, np.float32))
    y = np.asarray(y, np.float32)
    weight = np.asarray(weight, np.float32)
    mod_weight = np.asarray(mod_weight, np.float32)
    bias = np.asarray(bias, np.float32)

    # Style s[b,i] = sqrt(2) * leaky_relu(y @ (mod_weight * dlat^-0.5).T + bias).
    s = y @ (mod_weight.T * np.float32(DLAT ** -0.5))
    s = s + bias[None, :]
    s = np.where(s >= 0, s, LRELU_SLOPE * s).astype(np.float32) * np.float32(SQRT2)

    # Demod d[b,o] = rsqrt(sum_i s^2 * sum_t w^2 + eps) (exact refactoring;
    # fp32 tap-sums like the reference, tiny final reduction in fp64).
    w2 = np.einsum("oikl,oikl->oi", weight, weight, optimize=True)  # [C_OUT, C_IN]
    den = (s.astype(np.float64) ** 2) @ w2.T.astype(np.float64) + EPS
    dmod = (1.0 / np.sqrt(den)).astype(np.float32)

    if winograd is None:
        winograd = WINOGRAD
    if winograd:
        # G w along kw: [g0, (g0+g1+g2)/2, (g0-g1+g2)/2, g2] (exact in fp32).
        wt = weight.transpose(2, 3, 1, 0)               # [kh, kw, i, o]
        g0, g1, g2 = wt[:, 0], wt[:, 1], wt[:, 2]
        wtil = np.stack(
            [g0, (g0 + g1 + g2) * np.float32(0.5),
             (g0 - g1 + g2) * np.float32(0.5), g2], axis=1
        )                                               # [kh, xi, i, o]
        wT = np.ascontiguousarray(
            wtil.reshape(KS, 4, PCH_I, 128, C_OUT).transpose(2, 0, 1, 3, 4)
        )                                               # [kc, kh, xi, 128, o]
    else:
        # lhsT weights [kh,kw,i,o], in-chunk-major: [PCH_I, NTAPS, 128, C_OUT].
        wT = np.ascontiguousarray(
            weight.transpose(2, 3, 1, 0)
            .reshape(NTAPS, PCH_I, 128, C_OUT)
            .transpose(1, 0, 2, 3)
        )
    if mm_dtype == "bf16":
        import ml_dtypes

        wT = wT.astype(ml_dtypes.bfloat16)

    xg = x.reshape(B * PCH_I, 128, NPIX)                    # [32, 128, 1024]
    s_t = s.reshape(B, PCH_I, 128).transpose(0, 2, 1)       # [B, 128, 4]
    d_t = dmod.reshape(B, PCH_O, 128).transpose(0, 2, 1)    # [B, 128, 4]
    cols = [s_t, d_t] + ([-d_t] if winograd else [])
    sdm = np.ascontiguousarray(np.concatenate(cols, axis=2)).reshape(
        B * 128, PCH_I + (2 if winograd else 1) * PCH_O
    )

    return {"w": wT, "x": xg, "sdm": sdm}


def per_core_map(global_in, core):
    """Slice the global input dict into one core's input map (for CoreSim)."""
    return {
        "w": global_in["w"],
        "x": global_in["x"][core * PCH_I : (core + 1) * PCH_I],
        "sdm": global_in["sdm"][core * 128 : (core + 1) * 128],
    }


def _make_runner(nc):
    """Persistent jitted SPMD executor: weight replicated, rest batch-sharded,
    donated output zeros created on-device (nothing extra over the wire)."""
    import os

    import jax
    import jax.numpy as jnp
    from jax.sharding import Mesh, PartitionSpec
    from jax.experimental.shard_map import shard_map

    from concourse.bass2jax import (
        _bass_exec_p,
        install_neuronx_cc_hook,
        partition_id_tensor,
    )

    # Persist compiled executables (incl. the embedded NEFF) across processes;
    # the BIR is debug-scrubbed to be byte-stable, so this turns the multi-
    # minute first-call compile into a cache load. Respect an existing config.
    if jax.config.jax_compilation_cache_dir is None:
        jax.config.update(
            "jax_compilation_cache_dir",
            os.path.expanduser("~/.jax_bass_cache"),
        )
        jax.config.update("jax_persistent_cache_min_entry_size_bytes", -1)
        jax.config.update("jax_persistent_cache_min_compile_time_secs", 0.0)

    install_neuronx_cc_hook()

    partition_name = (
        nc.partition_id_tensor.name if nc.partition_id_tensor is not None else None
    )
    in_names: list = []
    out_names: list = []
    out_avals: list = []
    for alloc in nc.m.functions[0].allocations:
        if not isinstance(alloc, mybir.MemoryLocationSet):
            continue
        name = alloc.memorylocations[0].name
        if alloc.kind == "ExternalInput":
            if name != partition_name:
                in_names.append(name)
        elif alloc.kind == "ExternalOutput":
            out_names.append(name)
            out_avals.append(
                jax.core.ShapedArray(tuple(alloc.tensor_shape), mybir.dt.np(alloc.dtype))
            )
    all_in_names = list(in_names) + list(out_names)
    if partition_name is not None:
        all_in_names.append(partition_name)

    def _body(*args):
        operands = list(args)
        if partition_name is not None:
            operands.append(partition_id_tensor())
        outs = _bass_exec_p.bind(
            *operands,
            out_avals=tuple(out_avals),
            in_names=tuple(all_in_names),
            out_names=tuple(out_names),
            lowering_input_output_aliases=(),
            sim_require_finite=True,
            sim_require_nnan=True,
            nc=nc,
        )
        return tuple(outs)

    devices = [d for d in jax.devices() if d.platform != "cpu"][:N_CORES]
    if len(devices) < N_CORES:
        # e.g. the process pinned JAX_PLATFORMS=cpu — ask for the axon/neuron
        # backend explicitly.
        for plat in ("neuron", "axon"):
            try:
                devices = jax.devices(plat)[:N_CORES]
                break
            except RuntimeError:
                continue
    assert len(devices) == N_CORES, f"need {N_CORES} devices, got {len(devices)}"
    mesh = Mesh(np.asarray(devices), ("core",))
    spec_by_name = {n: PartitionSpec("core") for n in in_names}
    spec_by_name["w"] = PartitionSpec()          # replicated: one copy over the wire
    in_specs = tuple(spec_by_name[n] for n in in_names) + (
        (PartitionSpec("core"),) * len(out_names)
    )
    out_specs = (PartitionSpec("core"),) * len(out_names)
    fn = jax.jit(
        shard_map(
            _body, mesh=mesh, in_specs=in_specs, out_specs=out_specs, check_rep=False
        )
    )

    # Output "seed" buffers: the bass_exec custom call requires one parameter
    # per ExternalOutput. Our kernel writes every output element, so they only
    # need to exist, not be re-zeroed per call — create once on-device.
    from jax.sharding import NamedSharding

    def _mk_zeros():
        return tuple(
            jnp.zeros((N_CORES * a.shape[0],) + a.shape[1:], a.dtype)
            for a in out_avals
        )

    zeros_sh = tuple(
        NamedSharding(mesh, PartitionSpec("core")) for _ in out_avals
    )
    out_seeds = jax.jit(_mk_zeros, out_shardings=zeros_sh)()
    return fn, in_names, out_names, mesh, out_seeds


def get_runner(mm_dtype=MM_DTYPE):
    key = (mm_dtype, WINOGRAD)
    if key not in _RUNNER_CACHE:
        _RUNNER_CACHE[key] = _make_runner(get_nc(mm_dtype))
    return _RUNNER_CACHE[key]


def _w_device(wT, mesh):
    """Cache the replicated weight on-device across calls (keyed by content)."""
    import jax
    from jax.sharding import NamedSharding, PartitionSpec

    key = hashlib.blake2b(wT.tobytes(), digest_size=16).hexdigest()
    hit = _W_DEV_CACHE.get(key)
    if hit is None:
        sh = NamedSharding(mesh, PartitionSpec())
        _W_DEV_CACHE.clear()
        hit = _W_DEV_CACHE[key] = jax.device_put(wT, sh)
    return hit


def kernel(x, y, weight, mod_weight, bias):
    gin = prepare_inputs(x, y, weight, mod_weight, bias, MM_DTYPE)
    fn, in_names, out_names, mesh, out_seeds = get_runner(MM_DTYPE)
    gin["w"] = _w_device(gin["w"], mesh)
    outs = fn(*[gin[n] for n in in_names], *out_seeds)
    out = np.asarray(outs[out_names.index("o")])             # [8*512, 1024]
    return out.reshape(B, C_OUT, H, W).astype(np.float32, copy=False)

